# revision 1
# baseline (speedup 1.0000x reference)
"""TRN2 Bass kernel for nn_Attention_86260123173325.

Single-head attention over N=4096 tokens, feature dim HW=4096:
  q, k, v = x[:,0], x[:,1], x[:,2] reshaped to [4096, 4096]
  out = softmax(0.5 * q @ k.T) @ v

Sharding: q rows split across 8 cores (512 rows each); k, v replicated.
Host-side marshaling pre-transposes q and k into the PE-ready
contraction-major layouts (the PE reduces along the partition dim, so both
phase-1 operands need the feature dim on partitions); this removes ~1150
on-chip PE transposes + their PSUM->SBUF copies per core.

Per-core algorithm (matmuls in f32r = TF32-like, 4x the fp32 PE rate;
measured ~12.4-bit mantissa on HW, which keeps softmax argmax flips to
near-tied rows only => ~4e-3 absmax-relative output error):
  - Phase 1, per 128-row k block j:
    R^T[j,:] = k_j @ q^T via 32 accumulated f32r matmuls (R = raw q.k dots,
    dp = 0.5*R). Keep R^T in SBUF (fp32), and accumulate a row statistic
    W_i = sum_j exp(0.2*dp_ij - 40) via exp on ACT + ones^T @ E_t matmuls
    ([2,512] PSUM row, full-rate N=512). The -40 bias keeps W far below
    ~2^64 where the HW exp/f32r/ln chain was observed to break.
  - shift_i = 5*(ln(W_i) + 40) = LSE_{t=0.2}/t >= rowmax_i. Any per-row
    shift cancels in the final normalization, so exp(dp - shift) is an
    exact softmax numerator, can never overflow (dp - shift <= 0), and
    loses only terms below fp32 output resolution. Works for any data
    with |dp| <~ 420.
  - Pass 2: eT = exp(0.5*R - shift), f32r.
  - rowsum via E^T.T @ ones ([128,2] psum, f32r needs even free sizes)
  - Phase 2: O = (E @ v) * (1/rowsum), accumulating over j blocks in PSUM.
"""
import sys

sys.path.insert(0, "/opt/trn_rl_repo")

import numpy as np

import concourse.bass as bass
import concourse.tile as tile
from concourse import bacc, mybir
from concourse.bass_utils import run_bass_kernel_spmd

F32 = mybir.dt.float32
F32R = mybir.dt.float32r
EXP = mybir.ActivationFunctionType.Exp
LN = mybir.ActivationFunctionType.Ln

N_CORES = 8
N = 4096          # tokens (keys)
D = 4096          # feature dim (H*W)
M = N // N_CORES  # q rows per core = 512
NJ = N // 128     # 32 key blocks
ND = D // 128     # 32 feature blocks
NI = M // 128     # 4 q-row blocks per core
NDT = D // 512    # 8 output column tiles
T_STAT = 0.2      # stage-1 temperature: exp(t*dp - 40) = exp(0.1*R - 40)
STAT_BIAS = 40.0


def _build_nc():
    nc = bacc.Bacc(None, target_bir_lowering=False, debug=False)

    # qT[p, db, i] = q[i, db*128+p]; kT[jb, p, db, jj] = k[jb*128+jj, db*128+p]
    qT_dram = nc.dram_tensor("qT", [128, ND, M], F32R, kind="ExternalInput")
    kT_dram = nc.dram_tensor("kT", [NJ, 128, ND, 128], F32R, kind="ExternalInput")
    v_dram = nc.dram_tensor("v", [N, D], F32R, kind="ExternalInput")
    o_dram = nc.dram_tensor("o", [M, D], F32, kind="ExternalOutput")

    with tile.TileContext(nc) as tc:
        with tc.tile_pool(name="persist", bufs=1) as persist:
            # R^T storage, [j-within-block, j-block, i] (fp32, exact scores)
            s_sb = persist.tile([128, NJ, M], F32)

            # all-ones [128,128]: W-stat lhsT (output lands broadcast on all
            # 128 partitions); [:, 0:2] slices serve the rowsum matmuls
            # (f32r requires even free sizes on all matmul operands)
            ones_f = persist.tile([128, 128], F32, tag="ones_f")
            nc.vector.memset(ones_f[:], 1.0)
            ones_r = persist.tile([128, 128], F32R, tag="ones_r")
            nc.vector.tensor_copy(ones_r[:], ones_f[:])

            zero_b = persist.tile([128, 1], F32, tag="zero_b")
            nc.vector.memset(zero_b[:], 0.0)

            # stage-1 exp bias: keeps W = sum exp(0.2*dp - 40) well under
            # ~2^64, where the HW exp/f32r-matmul/ln chain was observed to
            # produce junk (rows with rowmax>232 went NaN without it)
            stat_b = persist.tile([128, 1], F32, tag="stat_b")
            nc.vector.memset(stat_b[:], -STAT_BIAS)

            sh2_bc = persist.tile([128, M], F32, tag="sh2_bc")
            rsum = persist.tile([128, NI], F32, tag="rsum")
            rinv = persist.tile([128, NI], F32, tag="rinv")

            # ---------------- phase 1: R^T blocks + W stats ----------------
            with (
                tc.tile_pool(name="qT", bufs=1) as qTpool,
                tc.tile_pool(name="kT", bufs=3) as kTpool,
                tc.tile_pool(name="ett", bufs=2) as etpool,
                tc.tile_pool(name="psS", bufs=2, space="PSUM") as psS,
                tc.tile_pool(name="psW", bufs=1, space="PSUM") as psWp,
            ):
                # 8 chunk tiles -> fine-grained deps; matmuls start after
                # the first 1MB chunk instead of the full 8.4MB load
                qT_parts = []
                for b in range(8):
                    qp = qTpool.tile([128, ND // 8, M], F32R, tag=f"qT{b}",
                                     name=f"qT{b}")
                    nc.scalar.dma_start(
                        out=qp[:], in_=qT_dram[:, b * (ND // 8):(b + 1) * (ND // 8), :]
                    )
                    qT_parts.append(qp)

                def qT_slice(dblk):
                    return qT_parts[dblk // (ND // 8)][:, dblk % (ND // 8), :]

                psW = psWp.tile([128, M], F32)
                for j in range(NJ):
                    kT = kTpool.tile([128, ND, 128], F32R, tag="kT",
                                     name=f"kT{j}")
                    nc.sync.dma_start(out=kT[:], in_=kT_dram[j])
                    ps = psS.tile([128, M], F32, tag="S", name=f"ps{j}")
                    for dblk in range(ND):
                        nc.tensor.matmul(
                            ps[:],
                            kT[:, dblk, :],
                            qT_slice(dblk),
                            start=(dblk == 0),
                            stop=(dblk == ND - 1),
                        )
                    # stash raw scores R^T (fp32)
                    nc.vector.tensor_copy(s_sb[:, j, :], ps[:])
                    # W stat: exp(0.1*R - 40) then ones^T @ E_t -> psW [2, M]
                    ett = etpool.tile([128, M], F32R, tag="ett", name=f"et{j}")
                    nc.scalar.activation(
                        out=ett[:], in_=ps[:], func=EXP,
                        bias=stat_b[:], scale=0.5 * T_STAT,
                    )
                    nc.tensor.matmul(
                        psW[:],
                        ones_r[:],
                        ett[:],
                        start=(j == 0),
                        stop=(j == NJ - 1),
                        skip_group_check=True,
                    )

                # sh2 = 2*shift = (2/t)*(lnW' + 40); psW rows are identical
                # (all-ones lhsT), so this lands already broadcast
                w_ln = persist.tile([128, M], F32, tag="w_ln")
                nc.scalar.activation(
                    out=w_ln[:], in_=psW[:], func=LN,
                    bias=zero_b[:], scale=1.0,
                )
                nc.vector.tensor_scalar(
                    sh2_bc[:], w_ln[:], 2.0 / T_STAT,
                    STAT_BIAS * 2.0 / T_STAT,
                    mybir.AluOpType.mult, mybir.AluOpType.add,
                )

            # phase-2 pools opened early: pre-issue the first v loads so
            # the PE has phase-2 work ready right after the exp pass starts
            _vstack = tc.tile_pool(name="vsrc", bufs=6)
            _ostack = tc.tile_pool(name="osb", bufs=6)
            _pstack = tc.tile_pool(name="psO", bufs=8, space="PSUM")
            vpool = _vstack.__enter__()
            opool = _ostack.__enter__()
            psO = _pstack.__enter__()
            v_pre = {}
            for jpre in range(6):
                vsb = vpool.tile([128, 512], F32R, tag="v", name=f"vp{jpre}")
                nc.sync.dma_start(
                    out=vsb[:], in_=v_dram[jpre * 128:(jpre + 1) * 128, 0:512]
                )
                v_pre[(0, jpre)] = vsb

            # ---------------- pass 2: eT = exp(0.5*R - shift) ------
            # (separate f32r tensor: the BIR verifier requires f32r matmul
            # inputs to be produced rounded)
            eTstack = tc.tile_pool(name="eTp", bufs=1)
            eTpool = eTstack.__enter__()
            eT_t = eTpool.tile([128, NJ, M], F32R, name="eT_t")
            with tc.tile_pool(name="tmp", bufs=3) as tmpool:
                for j in range(NJ):
                    tmp = tmpool.tile([128, M], F32, tag="tmp", name=f"tmp{j}")
                    nc.vector.tensor_sub(tmp[:], s_sb[:, j, :], sh2_bc[:])
                    nc.scalar.activation(
                        out=eT_t[:, j, :], in_=tmp[:],
                        func=EXP, bias=zero_b[:], scale=0.5,
                    )

            def eT(j, i0, i1):
                return eT_t[:, j, i0:i1]

            # ---------------- rowsums (share the 8-bank psO pool) ----------
            if True:
                for ib in range(NI):
                    pr = psO.tile([128, 2], F32, tag="o", name=f"pr{ib}")
                    for j in range(NJ):
                        nc.tensor.matmul(
                            pr[:],
                            eT(j, ib * 128, (ib + 1) * 128),
                            ones_r[:, 0:2],
                            start=(j == 0),
                            stop=(j == NJ - 1),
                        )
                    nc.vector.tensor_copy(rsum[:, ib:ib + 1], pr[:, 0:1])
                nc.vector.reciprocal(rinv[:], rsum[:])

            # ---------------- phase 2: O = (E @ v) * rinv ----------------
            if True:
                for dt in range(NDT):
                    pos = [
                        psO.tile([128, 512], F32, tag="o", name=f"po{dt}_{ib}")
                        for ib in range(NI)
                    ]
                    for j in range(NJ):
                        vsb = v_pre.pop((dt, j), None)
                        if vsb is None:
                            vsb = vpool.tile([128, 512], F32R, tag="v",
                                             name=f"v{dt}_{j}")
                            nc.sync.dma_start(
                                out=vsb[:],
                                in_=v_dram[j * 128:(j + 1) * 128,
                                           dt * 512:(dt + 1) * 512],
                            )
                        for ib in range(NI):
                            nc.tensor.matmul(
                                pos[ib][:],
                                eT(j, ib * 128, (ib + 1) * 128),
                                vsb[:],
                                start=(j == 0),
                                stop=(j == NJ - 1),
                            )
                    for ib in range(NI):
                        osb = opool.tile([128, 512], F32, tag="osb",
                                         name=f"ob{dt}_{ib}")
                        nc.vector.tensor_scalar_mul(
                            osb[:], pos[ib][:], rinv[:, ib:ib + 1]
                        )
                        nc.scalar.dma_start(
                            out=o_dram[ib * 128:(ib + 1) * 128,
                                       dt * 512:(dt + 1) * 512],
                            in_=osb[:],
                        )
            for st in (eTstack, _pstack, _ostack, _vstack):
                st.__exit__(None, None, None)

    nc.compile()
    return nc


_NC_CACHE = None


def _get_nc():
    global _NC_CACHE
    if _NC_CACHE is None:
        _NC_CACHE = _build_nc()
    return _NC_CACHE


def _make_in_maps(x: np.ndarray) -> list:
    x = np.asarray(x)
    n, c, h, w = x.shape
    assert (n, c, h * w) == (N, 3, D), f"unexpected shape {x.shape}"
    xr = np.ascontiguousarray(x.reshape(n, c, h * w).transpose(1, 0, 2))
    q_full, k, v = xr[0], xr[1], xr[2]
    # kT[jb, p, db, jj] = k[jb*128+jj, db*128+p] -- per-(jb) contiguous 2MB
    kT = np.ascontiguousarray(
        k.reshape(NJ, 128, ND, 128).transpose(0, 3, 2, 1)
    )
    in_maps = []
    for core in range(N_CORES):
        qc = q_full[core * M:(core + 1) * M]          # [M, D]
        # qT[p, db, i] = q[i, db*128+p]
        qTc = np.ascontiguousarray(
            qc.reshape(M, ND, 128).transpose(2, 1, 0)
        )
        in_maps.append({"qT": qTc, "kT": kT, "v": v})
    return in_maps


def kernel(x: np.ndarray) -> np.ndarray:
    nc = _get_nc()
    res = run_bass_kernel_spmd(nc, _make_in_maps(x), core_ids=list(range(N_CORES)))
    out = np.concatenate([r["o"] for r in res.results], axis=0)
    return out.astype(np.float32)

